# revision 53
# baseline (speedup 1.0000x reference)
"""Bass/Trainium2 kernel for conv-QKV multi-head attention.

Problem: x (2,5,640,32,32); 3x3 SAME conv projections Q/K/V (640->640);
8-head attention over N=1024 tokens per (b,m) crop, head_dim=80; output
projection (640x640) applied per (b,n,m); output (2,1024,3200).

Sharding: tensor-parallel by head. Core h computes the 240 conv output
channels for head h's q/k/v (channel order [q,k,v], two tiles of
128+112 rows), full attention for its head over all 10 crops, and a
partial output projection against w_proj[:, h*80:(h+1)*80]. The 8
partial outputs are summed on the host.

Conv: 1-D Winograd F(2,3) along W. The host pre-pads x to 34x34 bf16
images and pre-transforms the weights with G along kx (w -> 4 j-points
of 3 dy-taps). On device, DVE computes the 4 input combos U_j
(B^T along W, 4 tensor ops per channel tile), the PE accumulates
Y[co][j] = sum_{dy,ci} Wg^T U over 15 matmuls of 512 cols (16 w-tiles x
32 h rows) into one PSUM bank per (co,j), and DVE applies the inverse
transform A^T (2 outputs per tile, bias folded in) straight into the
qkv SBUF tiles. 120 conv matmuls per crop instead of the direct-conv
180 (1.5x fewer PE columns streamed).

Attention (per head, S^T layout so softmax-sum is a matmul row): S^T
matmuls and exp are split at 512-col granularity so PSUM banks recycle
at the rate ACT drains them; V is transposed on the PE into a single
1-bank PSUM staging tile and copied to the [V^T;1] operand with one
DVE op. The O matmul's ones-column emits the softmax row-sum; the
division happens on the host. Compute is bf16 with f32 PSUM.
"""

import numpy as np
import ml_dtypes
from contextlib import ExitStack

BS, MC, C, H, W = 2, 5, 640, 32, 32
NH, HD = 8, 80
N = H * W           # 1024
CROPS = BS * MC     # 10
CIT = C // 128      # 5 input-channel tiles
PH, PW = H + 2, W + 2   # 34x34 padded image
SCALE = HD ** -0.5
NCORES = 8
VB = 97             # V^T block: 80 v-dims, 16 zero, 1 ones (row sums)

_BF16 = ml_dtypes.bfloat16
_G = np.array([[1, 0, 0], [.5, .5, .5], [.5, -.5, .5], [0, 0, 1]])


def _build_graph():
    import concourse.bacc as bacc
    from concourse import bass, mybir, tile, masks

    f32 = mybir.dt.float32
    bf16 = mybir.dt.bfloat16
    Exp = mybir.ActivationFunctionType.Exp
    Ident = mybir.ActivationFunctionType.Identity
    ADD = mybir.AluOpType.add
    SUB = mybir.AluOpType.subtract
    MULT = mybir.AluOpType.mult

    nc = bacc.Bacc("TRN2", target_bir_lowering=False, debug=False,
                   num_devices=NCORES)

    UPL = 16 * PH   # 544 cols per U plane
    u_ext = nc.declare_dram_parameter("u", [CROPS, 128, CIT * 4 * UPL], bf16, isOutput=False)
    wg_ext = nc.declare_dram_parameter("wg", [CIT, 128, 4 * 3 * 256], bf16, isOutput=False)
    bqkv_ext = nc.declare_dram_parameter("bqkv", [256, 1], f32, isOutput=False)
    wproj_ext = nc.declare_dram_parameter("wproj", [HD, C], bf16, isOutput=False)
    out_ext = nc.declare_dram_parameter("out", [CROPS, C, N], bf16, isOutput=True)
    rsum_ext = nc.declare_dram_parameter("rsum", [CROPS, 1, N], f32, isOutput=True)

    with tile.TileContext(nc) as tc, ExitStack() as ctx:
        const = ctx.enter_context(tc.tile_pool(name="const", bufs=1))
        sb = ctx.enter_context(tc.tile_pool(name="sb", bufs=2))
        psum = ctx.enter_context(tc.tile_pool(name="psum", bufs=2, space="PSUM"))

        ident = const.tile([128, 128], bf16, tag="ident")
        masks.make_identity(nc, ident[:])

        # Winograd weights, one tile (and one DMA) per input-channel tile.
        wg_sb = [const.tile([128, 4 * 3 * 256], bf16, tag=f"wg{t}", name=f"wg{t}")
                 for t in range(CIT)]
        bias_a = const.tile([128, 1], f32, tag="bias_a")
        nc.sync.dma_start(bias_a[:], bqkv_ext[0:128])
        bias_b = const.tile([128, 1], f32, tag="bias_b")
        nc.sync.dma_start(bias_b[:], bqkv_ext[128:256])
        wp_sb = const.tile([HD, C], bf16, tag="wproj")
        nc.sync.dma_start(wp_sb[:], wproj_ext[:])

        # Double-buffered Winograd input planes U_j (B^T-transformed on the
        # host — a fixed linear relayout of x, so it costs no device time).
        # One big tile per crop parity -> one DMA trigger per crop.
        ubig = [const.tile([128, CIT * 4 * UPL], bf16, tag=f"u{s}", name=f"u{s}")
                for s in range(2)]

        def uplane(su, t, j):
            return ubig[su][:, (t * 4 + j) * UPL:(t * 4 + j + 1) * UPL]
        pT = [[const.tile([128, N], bf16, tag=f"pT{s}_{kb}", name=f"pT{s}_{kb}")
               for kb in range(8)] for s in range(2)]
        # vT blocks of 97 columns: [0:80] = v^T, [80:96] zero, col 96 = ones
        # (the O matmul then also emits the softmax row-sum as output row 96).
        vT2 = [const.tile([128, 8 * VB], bf16, tag=f"vt{s}", name=f"vt{s}")
               for s in range(2)]
        for s in range(2):
            nc.vector.memset(vT2[s][:], 0.0)
            for kb in range(8):
                nc.vector.memset(vT2[s][:, kb * VB + 96: kb * VB + VB], 1.0)

        def xload(c):
            nc.sync.dma_start(ubig[c % 2][:], u_ext[c])

        qkv_of = {}

        def conv_phase(c, fillers=(), mid=None):
            """Winograd conv: per co, 15 weight-waves of 4 j-interleaved
            matmuls; qkv_a = [q80, v0:48], qkv_b = [k80, v48:80, 16 dead].
            Filler units (previous crop's attention front half) are spliced
            between waves; `mid` (the 2-crops-ago projection) runs between
            the co stretches so its PE work hides the inverse-transform
            latency that must free the wY banks."""
            su = c % 2
            qkv_a = sb.tile([128, N], bf16, tag="qkv_a")
            qkv_b = sb.tile([128, N], bf16, tag="qkv_b")
            qkv_of[c] = (qkv_a, qkv_b)
            fillers = list(fillers)
            for co in range(2):
                # j-interleaved emission: consecutive matmuls rotate over the
                # 4 wY PSUM banks (back-to-back accumulation into a single
                # bank runs ~1.2x slower than bank-alternating).
                ys = [psum.tile([128, 512], f32, tag=f"wY{j}", bufs=1,
                                name=f"pc{j}")
                      for j in range(4)]
                ki = 0
                for dy in range(3):
                    for t in range(CIT):
                        for j in range(4):
                            ur = uplane(su, t, j).rearrange(
                                "p (h tc) -> p h tc", tc=16)
                            base = j * 768 + dy * 256 + co * 128
                            lhsT = wg_sb[t][:, base: base + 128]
                            nc.tensor.matmul(ys[j][:], lhsT,
                                             ur[:, dy:dy + 32, :],
                                             start=(ki == 0), stop=(ki == 14))
                        ki += 1
                        if ki % 3 == 0 and ki < 15 and fillers:
                            fillers.pop(0)()
                # Inverse transform A^T (b=0: Y0+Y1+Y2, b=1: Y1-Y2-Y3) with
                # bias folded in, written straight into qkv[:, h, 2tc+b].
                q = qkv_a if co == 0 else qkv_b
                bias = bias_a if co == 0 else bias_b
                # qkv stays in n'-order (w-parity, h, w2) so each b-output is
                # a contiguous 512-col slab; attention is n-permutation-
                # invariant and the host un-permutes at the end.
                # Each DVE op may read at most ONE PSUM operand: stage Y1
                # through ACT (it has headroom), combos on DVE, all flat.
                y1s = sb.tile([128, 512], f32, tag="y1s")
                nc.scalar.activation(y1s[:], ys[1][:], Ident)
                t0 = sb.tile([128, 512], f32, tag="tinv")
                nc.vector.tensor_add(out=t0[:], in0=y1s[:], in1=ys[0][:])
                nc.vector.scalar_tensor_tensor(
                    out=q[:, 0:512], in0=t0[:], scalar=bias[:],
                    in1=ys[2][:], op0=ADD, op1=ADD)
                t1 = sb.tile([128, 512], f32, tag="tinv")
                nc.vector.scalar_tensor_tensor(
                    out=t1[:], in0=y1s[:], scalar=bias[:], in1=ys[2][:],
                    op0=ADD, op1=SUB)
                nc.vector.tensor_sub(out=q[:, 512:1024], in0=t1[:],
                                     in1=ys[3][:])
                if co == 0:
                    if fillers:
                        fillers.pop(0)()
                    if mid is not None:
                        mid()
            for f in fillers:
                f()

        ot_of = {}

        def attn_fillers(c, perkb_copy=False):
            """Per-kb PE units of crop c's attention front half: vT
            transpose into a 1-bank PSUM staging tile + S^T matmuls with
            per-512-col exp. Final unit copies the staged V^T out (or each
            unit copies its own block when `perkb_copy`, so the epilogue
            can consume vt incrementally)."""
            s = c % 2
            qkv_a, qkv_b = qkv_of[c]
            v_sb = sb.tile([HD, N], bf16, tag="v_sb")
            nc.sync.dma_start(v_sb[0:48, :], qkv_a[80:128, :])
            nc.sync.dma_start(v_sb[48:80, :], qkv_b[80:112, :])
            # staging stride 98 (not 97) so each bf16 block is 4B-aligned
            TVB = 98
            tva = psum.tile([128, 8 * TVB], bf16, tag="tv", bufs=1)
            vt = vT2[s]

            def unit(kb):
                def f():
                    nc.tensor.transpose(tva[:, kb * TVB: kb * TVB + HD],
                                        v_sb[:, kb * 128:(kb + 1) * 128],
                                        ident[0:HD, 0:HD])
                    if perkb_copy:
                        nc.vector.tensor_copy(
                            vt[:, kb * VB: kb * VB + HD],
                            tva[:, kb * TVB: kb * TVB + HD])
                    for h2 in range(2):
                        st = psum.tile([128, 512], f32, tag="st", bufs=3)
                        nc.tensor.matmul(
                            st[:],
                            qkv_b[0:HD, kb * 128:(kb + 1) * 128],
                            qkv_a[0:HD, h2 * 512:(h2 + 1) * 512],
                            start=True, stop=True)
                        nc.scalar.activation(
                            pT[s][kb][:, h2 * 512:(h2 + 1) * 512], st[:],
                            Exp, scale=SCALE)
                return f

            def vt_copy():
                if perkb_copy:
                    return
                dst = vt[:].rearrange("p (kb vb) -> p kb vb", vb=VB)[:, :, 0:HD]
                src = tva[:].rearrange("p (kb vb) -> p kb vb", vb=TVB)[:, :, 0:HD]
                nc.vector.tensor_copy(dst, src)

            return [unit(kb) for kb in range(8)] + [vt_copy]

        def attn_tail(c):
            """O matmul + output staging for crop c (after its fillers).
            [O^T; rowsum] = [V^T; 1]^T P^T  (row 96 = softmax sums)."""
            s = c % 2
            qkv_of.pop(c)
            vt = vT2[s]
            ot = sb.tile([HD, N], bf16, tag="ot")
            rrow = sb.tile([1, N], f32, tag="rrow")
            # kb-outer / h2-inner: consecutive matmuls share the vt weight
            # block and alternate the two accumulation banks
            po = [psum.tile([VB, 512], f32, tag="st", name=f"po{h2}", bufs=3)
                  for h2 in range(2)]
            for kb in range(8):
                for h2 in range(2):
                    nc.tensor.matmul(
                        po[h2][:], vt[:, kb * VB:(kb + 1) * VB],
                        pT[s][kb][:, h2 * 512:(h2 + 1) * 512],
                        start=(kb == 0), stop=(kb == 7))
            for h2 in range(2):
                nc.scalar.activation(ot[:, h2 * 512:(h2 + 1) * 512],
                                     po[h2][0:HD, :], Ident)
                nc.vector.tensor_copy(rrow[:, h2 * 512:(h2 + 1) * 512],
                                      po[h2][96:97, :])
            ot_of[c] = ot
            nc.sync.dma_start(rsum_ext[c], rrow[:])

        def proj_phase(c, fillers=()):
            ot = ot_of.pop(c)
            fillers = list(fillers)
            for dt in range(5):
                osb = sb.tile([128, N], bf16, tag="osb")
                for h2 in range(2):
                    pp = psum.tile([128, 512], f32, tag="st", bufs=3)
                    nc.tensor.matmul(pp[:], wp_sb[:, dt * 128:(dt + 1) * 128],
                                     ot[:, h2 * 512:(h2 + 1) * 512],
                                     start=True, stop=True)
                    if h2 == 0:
                        nc.scalar.activation(osb[:, 0:512], pp[:], Ident)
                    else:
                        nc.vector.tensor_copy(osb[:, 512:1024], pp[:])
                nc.sync.dma_start(out_ext[c, dt * 128:(dt + 1) * 128, :], osb[:])
                for _ in range(2):
                    if fillers:
                        fillers.pop(0)()
            for f in fillers:
                f()

        # Software-pipelined emission: crop c's S^T/vT units are spliced
        # into crop c+1's conv stream (their ACT/DVE consumers overlap the
        # conv matmuls), then O(c) and proj(c-1) follow.
        # Prologue DMAs interleaved per channel-tile so conv(0)'s first
        # waves can start as soon as their own weight/U chunks land.
        for t in range(CIT):
            nc.sync.dma_start(wg_sb[t][:], wg_ext[t])
            nc.sync.dma_start(ubig[0][:, t * 4 * UPL:(t + 1) * 4 * UPL],
                              u_ext[0][:, t * 4 * UPL:(t + 1) * 4 * UPL])
        xload(1)
        # PE warmup: dummy matmuls on the identity while the first crop
        # loads, so conv(0) starts at full clock.
        warm = psum.tile([128, 128], f32, tag="st", bufs=3)
        for _ in range(100):
            nc.tensor.matmul(warm[:], ident[:], ident[:], start=True, stop=True)
        conv_phase(0)
        for c in range(CROPS):
            if c + 2 < CROPS:
                xload(c + 2)
            if c + 1 < CROPS:
                fills = attn_fillers(c)
                conv_phase(c + 1, fills,
                           mid=(lambda cc=c: proj_phase(cc - 1)) if c >= 1
                           else None)
                attn_tail(c)
            else:
                # epilogue: no next conv to hide in. Interleave the last
                # crop's S units with proj(c-1) matmuls AND with its own
                # per-kb O accumulation so the PE never waits on the exp
                # chain longer than necessary.
                s = c % 2
                vt = vT2[s]
                ot = sb.tile([HD, N], bf16, tag="ot")
                rrow = sb.tile([1, N], f32, tag="rrow")
                po = [psum.tile([VB, 512], f32, tag="st", name=f"poe{h2}",
                                bufs=3) for h2 in range(2)]
                proj_mm = []
                otp = ot_of.pop(c - 1)
                for dt in range(5):
                    osb = sb.tile([128, N], bf16, tag="osb")
                    def pjdt(dt=dt, osb=osb):
                        for h2 in range(2):
                            pp = psum.tile([128, 512], f32, tag="st", bufs=3)
                            nc.tensor.matmul(
                                pp[:], wp_sb[:, dt * 128:(dt + 1) * 128],
                                otp[:, h2 * 512:(h2 + 1) * 512],
                                start=True, stop=True)
                            if h2 == 0:
                                nc.scalar.activation(osb[:, 0:512], pp[:], Ident)
                            else:
                                nc.vector.tensor_copy(osb[:, 512:1024], pp[:])
                        nc.sync.dma_start(
                            out_ext[c - 1, dt * 128:(dt + 1) * 128, :], osb[:])
                    proj_mm.append(pjdt)
                units = attn_fillers(c, perkb_copy=True)[:8]

                def o_mm(kb):
                    for h2 in range(2):
                        nc.tensor.matmul(
                            po[h2][:], vt[:, kb * VB:(kb + 1) * VB],
                            pT[s][kb][:, h2 * 512:(h2 + 1) * 512],
                            start=(kb == 0), stop=(kb == 7))

                # O trails the S/exp producer by one kb so the PE never
                # waits on a just-issued exp
                for kb in range(8):
                    units[kb]()
                    if kb >= 1:
                        o_mm(kb - 1)
                    if kb < 5:
                        proj_mm[kb]()
                o_mm(7)
                for h2 in range(2):
                    nc.scalar.activation(ot[:, h2 * 512:(h2 + 1) * 512],
                                         po[h2][0:HD, :], Ident)
                    nc.vector.tensor_copy(rrow[:, h2 * 512:(h2 + 1) * 512],
                                          po[h2][96:97, :])
                ot_of[c] = ot
                nc.sync.dma_start(rsum_ext[c], rrow[:])
        proj_phase(CROPS - 1)

    nc.compile()
    return nc


def _host_inputs(x, wq, bq, wk, bk, wv, bv, w_proj):
    """Per-core input maps; conv output channels ordered [q, k, v]."""
    # Shared across cores: the Winograd F(2,3)-along-W input transform
    # U_j = B^T-combos of the padded bf16 image, computed on the host.
    # Planes are [h(34), w2(16)]; SBUF layout [128, t, j, 544].
    xf = np.asarray(x, dtype=np.float32).reshape(CROPS, C, H, W)
    xpad = np.zeros((CROPS, C, PH, PW), np.float32)
    xpad[:, :, 1:1 + H, 1:1 + W] = xf.astype(_BF16).astype(np.float32)
    ev0 = xpad[..., 0:32:2]
    od0 = xpad[..., 1:33:2]
    ev1 = xpad[..., 2:34:2]
    od1 = xpad[..., 3:35:2]
    u = np.stack([ev0 - ev1, od0 + ev1, ev1 - od0, od0 - od1], axis=2)
    u = u.reshape(CROPS, CIT, 128, 4, PH * 16).transpose(0, 2, 1, 3, 4)
    u = np.ascontiguousarray(u).reshape(CROPS, 128, CIT * 4 * PH * 16)
    u = u.astype(_BF16)

    in_maps = []
    for h in range(NCORES):
        sl = slice(h * HD, (h + 1) * HD)
        zpad = np.zeros((16,) + wq.shape[1:], wq.dtype)
        w_cat = np.concatenate(
            [wq[sl], wv[sl][:48], wk[sl], wv[sl][48:], zpad], axis=0)  # [256,...]
        # G-transform along kx: laid out [CIT, 128, j, dy, co] = [5, 128, 2304]
        wg = np.einsum('jk,ocdk->cjdo', _G, w_cat.astype(np.float64))
        wg = np.ascontiguousarray(
            wg.reshape(CIT, 128, 4, 3, 256)).reshape(CIT, 128, 4 * 3 * 256)
        b_cat = np.concatenate(
            [bq[sl], bv[sl][:48], bk[sl], bv[sl][48:],
             np.zeros(16, bq.dtype)]).reshape(256, 1)
        wpT = np.ascontiguousarray(w_proj[:, sl].T)  # [80, 640]
        in_maps.append({
            "u": u,
            "wg": wg.astype(_BF16),
            "bqkv": b_cat.astype(np.float32),
            "wproj": wpT.astype(_BF16),
        })
    return in_maps


def _host_reduce(results, b_proj):
    acc = np.zeros((CROPS, C, N), np.float32)
    for r in results:
        acc += r["out"].astype(np.float32) / r["rsum"]
    # un-permute n' = (w-parity, h, w2) back to n = (h, w)
    acc = np.ascontiguousarray(
        acc.reshape(CROPS, C, 2, H, W // 2).transpose(0, 1, 3, 4, 2)
    ).reshape(CROPS, C, N)
    o = acc.reshape(BS, MC, C, N).transpose(0, 3, 1, 2)  # [b, n, m, dout]
    o = o + b_proj[None, None, None, :].astype(np.float32)
    return np.ascontiguousarray(o.reshape(BS, N, MC * C), dtype=np.float32)


_NC_CACHE = {}


def kernel(x, wq, bq, wk, bk, wv, bv, w_proj, b_proj, _run_kwargs=None):
    from concourse.bass_utils import run_bass_kernel_spmd

    if "nc" not in _NC_CACHE:
        _NC_CACHE["nc"] = _build_graph()
    nc = _NC_CACHE["nc"]
    in_maps = _host_inputs(x, wq, bq, wk, bk, wv, bv, w_proj)
    res = run_bass_kernel_spmd(nc, in_maps, core_ids=list(range(NCORES)),
                               **(_run_kwargs or {}))
    out = _host_reduce(res.results, np.asarray(b_proj))
    if _run_kwargs:
        _NC_CACHE["last_result"] = res
    return out


# revision 54
# speedup vs baseline: 1.0324x; 1.0324x over previous
"""Bass/Trainium2 kernel for conv-QKV multi-head attention.

Problem: x (2,5,640,32,32); 3x3 SAME conv projections Q/K/V (640->640);
8-head attention over N=1024 tokens per (b,m) crop, head_dim=80; output
projection (640x640) applied per (b,n,m); output (2,1024,3200).

Sharding: tensor-parallel by head. Core h computes the 240 conv output
channels for head h's q/k/v (channel order [q,k,v], two tiles of
128+112 rows), full attention for its head over all 10 crops, and a
partial output projection against w_proj[:, h*80:(h+1)*80]. The 8
partial outputs are summed on the host.

Conv: 1-D Winograd F(2,3) along W. The host pre-pads x to 34x34 bf16
images and pre-transforms the weights with G along kx (w -> 4 j-points
of 3 dy-taps). On device, DVE computes the 4 input combos U_j
(B^T along W, 4 tensor ops per channel tile), the PE accumulates
Y[co][j] = sum_{dy,ci} Wg^T U over 15 matmuls of 512 cols (16 w-tiles x
32 h rows) into one PSUM bank per (co,j), and DVE applies the inverse
transform A^T (2 outputs per tile, bias folded in) straight into the
qkv SBUF tiles. 120 conv matmuls per crop instead of the direct-conv
180 (1.5x fewer PE columns streamed).

Attention (per head, S^T layout so softmax-sum is a matmul row): S^T
matmuls and exp are split at 512-col granularity so PSUM banks recycle
at the rate ACT drains them; V is transposed on the PE into a single
1-bank PSUM staging tile and copied to the [V^T;1] operand with one
DVE op. The O matmul's ones-column emits the softmax row-sum; the
division happens on the host. Compute is bf16 with f32 PSUM.
"""

import numpy as np
import ml_dtypes
from contextlib import ExitStack

BS, MC, C, H, W = 2, 5, 640, 32, 32
NH, HD = 8, 80
N = H * W           # 1024
CROPS = BS * MC     # 10
CIT = C // 128      # 5 input-channel tiles
PH, PW = H + 2, W + 2   # 34x34 padded image
SCALE = HD ** -0.5
NCORES = 8
VB = 97             # V^T block: 80 v-dims, 16 zero, 1 ones (row sums)

_BF16 = ml_dtypes.bfloat16
_G = np.array([[1, 0, 0], [.5, .5, .5], [.5, -.5, .5], [0, 0, 1]])


def _build_graph():
    import concourse.bacc as bacc
    from concourse import bass, mybir, tile, masks

    f32 = mybir.dt.float32
    bf16 = mybir.dt.bfloat16
    Exp = mybir.ActivationFunctionType.Exp
    Ident = mybir.ActivationFunctionType.Identity
    ADD = mybir.AluOpType.add
    SUB = mybir.AluOpType.subtract
    MULT = mybir.AluOpType.mult

    nc = bacc.Bacc("TRN2", target_bir_lowering=False, debug=False,
                   num_devices=NCORES)

    UPL = 16 * PH   # 544 cols per U plane
    u_ext = nc.declare_dram_parameter("u", [CROPS, 128, CIT * 4 * UPL], bf16, isOutput=False)
    wg_ext = nc.declare_dram_parameter("wg", [CIT, 128, 4 * 3 * 256], bf16, isOutput=False)
    bqkv_ext = nc.declare_dram_parameter("bqkv", [256, 1], f32, isOutput=False)
    wproj_ext = nc.declare_dram_parameter("wproj", [HD, C], bf16, isOutput=False)
    out_ext = nc.declare_dram_parameter("out", [CROPS, C, N], bf16, isOutput=True)
    rsum_ext = nc.declare_dram_parameter("rsum", [CROPS, 1, N], f32, isOutput=True)

    with tile.TileContext(nc) as tc, ExitStack() as ctx:
        const = ctx.enter_context(tc.tile_pool(name="const", bufs=1))
        sb = ctx.enter_context(tc.tile_pool(name="sb", bufs=2))
        psum = ctx.enter_context(tc.tile_pool(name="psum", bufs=2, space="PSUM"))

        ident = const.tile([128, 128], bf16, tag="ident")
        masks.make_identity(nc, ident[:])

        # Winograd weights, one tile (and one DMA) per input-channel tile.
        wg_sb = [const.tile([128, 4 * 3 * 256], bf16, tag=f"wg{t}", name=f"wg{t}")
                 for t in range(CIT)]
        bias_a = const.tile([128, 1], f32, tag="bias_a")
        nc.sync.dma_start(bias_a[:], bqkv_ext[0:128])
        bias_b = const.tile([128, 1], f32, tag="bias_b")
        nc.sync.dma_start(bias_b[:], bqkv_ext[128:256])
        wp_sb = const.tile([HD, C], bf16, tag="wproj")
        nc.sync.dma_start(wp_sb[:], wproj_ext[:])

        # Double-buffered Winograd input planes U_j (B^T-transformed on the
        # host — a fixed linear relayout of x, so it costs no device time).
        # One big tile per crop parity -> one DMA trigger per crop.
        ubig = [const.tile([128, CIT * 4 * UPL], bf16, tag=f"u{s}", name=f"u{s}")
                for s in range(2)]

        def uplane(su, t, j):
            return ubig[su][:, (t * 4 + j) * UPL:(t * 4 + j + 1) * UPL]
        pT = [[const.tile([128, N], bf16, tag=f"pT{s}_{kb}", name=f"pT{s}_{kb}")
               for kb in range(8)] for s in range(2)]
        # vT blocks of 97 columns: [0:80] = v^T, [80:96] zero, col 96 = ones
        # (the O matmul then also emits the softmax row-sum as output row 96).
        vT2 = [const.tile([128, 8 * VB], bf16, tag=f"vt{s}", name=f"vt{s}")
               for s in range(2)]
        for s in range(2):
            nc.vector.memset(vT2[s][:], 0.0)
            for kb in range(8):
                nc.vector.memset(vT2[s][:, kb * VB + 96: kb * VB + VB], 1.0)

        def xload(c):
            nc.sync.dma_start(ubig[c % 2][:], u_ext[c])

        qkv_of = {}

        def conv_phase(c, fillers=(), mid=None):
            """Winograd conv: per co, 15 weight-waves of 4 j-interleaved
            matmuls; qkv_a = [q80, v0:48], qkv_b = [k80, v48:80, 16 dead].
            Filler units (previous crop's attention front half) are spliced
            between waves; `mid` (the 2-crops-ago projection) runs between
            the co stretches so its PE work hides the inverse-transform
            latency that must free the wY banks."""
            su = c % 2
            qkv_a = sb.tile([128, N], bf16, tag="qkv_a")
            qkv_b = sb.tile([128, N], bf16, tag="qkv_b")
            qkv_of[c] = (qkv_a, qkv_b)
            fillers = list(fillers)
            for co in range(2):
                # j-interleaved emission: consecutive matmuls rotate over the
                # 4 wY PSUM banks (back-to-back accumulation into a single
                # bank runs ~1.2x slower than bank-alternating).
                ys = [psum.tile([128, 512], f32, tag=f"wY{j}", bufs=1,
                                name=f"pc{j}")
                      for j in range(4)]
                ki = 0
                for dy in range(3):
                    for t in range(CIT):
                        for j in range(4):
                            ur = uplane(su, t, j).rearrange(
                                "p (h tc) -> p h tc", tc=16)
                            base = j * 768 + dy * 256 + co * 128
                            lhsT = wg_sb[t][:, base: base + 128]
                            nc.tensor.matmul(ys[j][:], lhsT,
                                             ur[:, dy:dy + 32, :],
                                             start=(ki == 0), stop=(ki == 14))
                        ki += 1
                        if ki % 3 == 0 and ki < 15 and fillers:
                            fillers.pop(0)()
                # Inverse transform A^T (b=0: Y0+Y1+Y2, b=1: Y1-Y2-Y3) with
                # bias folded in, written straight into qkv[:, h, 2tc+b].
                q = qkv_a if co == 0 else qkv_b
                bias = bias_a if co == 0 else bias_b
                # qkv stays in n'-order (w-parity, h, w2) so each b-output is
                # a contiguous 512-col slab; attention is n-permutation-
                # invariant and the host un-permutes at the end.
                # Each DVE op may read at most ONE PSUM operand: stage Y1
                # through ACT (it has headroom), combos on DVE, all flat.
                y1s = sb.tile([128, 512], f32, tag="y1s")
                nc.scalar.activation(y1s[:], ys[1][:], Ident)
                t0 = sb.tile([128, 512], f32, tag="tinv")
                nc.vector.tensor_add(out=t0[:], in0=y1s[:], in1=ys[0][:])
                nc.vector.scalar_tensor_tensor(
                    out=q[:, 0:512], in0=t0[:], scalar=bias[:],
                    in1=ys[2][:], op0=ADD, op1=ADD)
                t1 = sb.tile([128, 512], f32, tag="tinv")
                nc.vector.scalar_tensor_tensor(
                    out=t1[:], in0=y1s[:], scalar=bias[:], in1=ys[2][:],
                    op0=ADD, op1=SUB)
                nc.vector.tensor_sub(out=q[:, 512:1024], in0=t1[:],
                                     in1=ys[3][:])
                if co == 0:
                    if fillers:
                        fillers.pop(0)()
                    if mid is not None:
                        mid()
            for f in fillers:
                f()

        ot_of = {}

        def attn_fillers(c):
            """Per-kb PE units of crop c's attention front half: vT
            transpose into a 1-bank PSUM staging tile + S^T matmuls with
            per-512-col exp. Final unit copies the staged V^T out."""
            s = c % 2
            qkv_a, qkv_b = qkv_of[c]
            v_sb = sb.tile([HD, N], bf16, tag="v_sb")
            nc.sync.dma_start(v_sb[0:48, :], qkv_a[80:128, :])
            nc.sync.dma_start(v_sb[48:80, :], qkv_b[80:112, :])
            # staging stride 98 (not 97) so each bf16 block is 4B-aligned
            TVB = 98
            tva = psum.tile([128, 8 * TVB], bf16, tag="tv", bufs=1)
            vt = vT2[s]

            def unit(kb):
                def f():
                    nc.tensor.transpose(tva[:, kb * TVB: kb * TVB + HD],
                                        v_sb[:, kb * 128:(kb + 1) * 128],
                                        ident[0:HD, 0:HD])
                    for h2 in range(2):
                        st = psum.tile([128, 512], f32, tag="st", bufs=3)
                        nc.tensor.matmul(
                            st[:],
                            qkv_b[0:HD, kb * 128:(kb + 1) * 128],
                            qkv_a[0:HD, h2 * 512:(h2 + 1) * 512],
                            start=True, stop=True)
                        nc.scalar.activation(
                            pT[s][kb][:, h2 * 512:(h2 + 1) * 512], st[:],
                            Exp, scale=SCALE)
                return f

            def vt_copy():
                dst = vt[:].rearrange("p (kb vb) -> p kb vb", vb=VB)[:, :, 0:HD]
                src = tva[:].rearrange("p (kb vb) -> p kb vb", vb=TVB)[:, :, 0:HD]
                nc.vector.tensor_copy(dst, src)

            return [unit(kb) for kb in range(8)] + [vt_copy]

        def attn_tail(c):
            """O matmul + output staging for crop c (after its fillers).
            [O^T; rowsum] = [V^T; 1]^T P^T  (row 96 = softmax sums)."""
            s = c % 2
            qkv_of.pop(c)
            vt = vT2[s]
            ot = sb.tile([HD, N], bf16, tag="ot")
            rrow = sb.tile([1, N], f32, tag="rrow")
            # kb-outer / h2-inner: consecutive matmuls share the vt weight
            # block and alternate the two accumulation banks
            po = [psum.tile([VB, 512], f32, tag="st", name=f"po{h2}", bufs=3)
                  for h2 in range(2)]
            for kb in range(8):
                for h2 in range(2):
                    nc.tensor.matmul(
                        po[h2][:], vt[:, kb * VB:(kb + 1) * VB],
                        pT[s][kb][:, h2 * 512:(h2 + 1) * 512],
                        start=(kb == 0), stop=(kb == 7))
            for h2 in range(2):
                nc.scalar.activation(ot[:, h2 * 512:(h2 + 1) * 512],
                                     po[h2][0:HD, :], Ident)
                nc.vector.tensor_copy(rrow[:, h2 * 512:(h2 + 1) * 512],
                                      po[h2][96:97, :])
            ot_of[c] = ot
            nc.sync.dma_start(rsum_ext[c], rrow[:])

        def proj_phase(c, fillers=()):
            ot = ot_of.pop(c)
            fillers = list(fillers)
            for dt in range(5):
                osb = sb.tile([128, N], bf16, tag="osb")
                for h2 in range(2):
                    pp = psum.tile([128, 512], f32, tag="st", bufs=3)
                    nc.tensor.matmul(pp[:], wp_sb[:, dt * 128:(dt + 1) * 128],
                                     ot[:, h2 * 512:(h2 + 1) * 512],
                                     start=True, stop=True)
                    if h2 == 0:
                        nc.scalar.activation(osb[:, 0:512], pp[:], Ident)
                    else:
                        nc.vector.tensor_copy(osb[:, 512:1024], pp[:])
                nc.sync.dma_start(out_ext[c, dt * 128:(dt + 1) * 128, :], osb[:])
                for _ in range(2):
                    if fillers:
                        fillers.pop(0)()
            for f in fillers:
                f()

        # Software-pipelined emission: crop c's S^T/vT units are spliced
        # into crop c+1's conv stream (their ACT/DVE consumers overlap the
        # conv matmuls), then O(c) and proj(c-1) follow.
        # Prologue DMAs interleaved per channel-tile so conv(0)'s first
        # waves can start as soon as their own weight/U chunks land.
        for t in range(CIT):
            nc.sync.dma_start(wg_sb[t][:], wg_ext[t])
            nc.sync.dma_start(ubig[0][:, t * 4 * UPL:(t + 1) * 4 * UPL],
                              u_ext[0][:, t * 4 * UPL:(t + 1) * 4 * UPL])
        xload(1)
        # PE warmup: dummy matmuls on the identity while the first crop
        # loads, so conv(0) starts at full clock.
        warm = psum.tile([128, 128], f32, tag="st", bufs=3)
        for _ in range(100):
            nc.tensor.matmul(warm[:], ident[:], ident[:], start=True, stop=True)
        conv_phase(0)
        for c in range(CROPS):
            if c + 2 < CROPS:
                xload(c + 2)
            fills = attn_fillers(c)
            if c + 1 < CROPS:
                conv_phase(c + 1, fills,
                           mid=(lambda cc=c: proj_phase(cc - 1)) if c >= 1
                           else None)
                attn_tail(c)
            else:
                # epilogue: last crop's S^T/vT units hide inside proj(c-1)
                proj_phase(c - 1, fills)
                attn_tail(c)
        proj_phase(CROPS - 1)

    nc.compile()
    return nc


def _host_inputs(x, wq, bq, wk, bk, wv, bv, w_proj):
    """Per-core input maps; conv output channels ordered [q, k, v]."""
    # Shared across cores: the Winograd F(2,3)-along-W input transform
    # U_j = B^T-combos of the padded bf16 image, computed on the host.
    # Planes are [h(34), w2(16)]; SBUF layout [128, t, j, 544].
    xf = np.asarray(x, dtype=np.float32).reshape(CROPS, C, H, W)
    xpad = np.zeros((CROPS, C, PH, PW), np.float32)
    xpad[:, :, 1:1 + H, 1:1 + W] = xf.astype(_BF16).astype(np.float32)
    ev0 = xpad[..., 0:32:2]
    od0 = xpad[..., 1:33:2]
    ev1 = xpad[..., 2:34:2]
    od1 = xpad[..., 3:35:2]
    u = np.stack([ev0 - ev1, od0 + ev1, ev1 - od0, od0 - od1], axis=2)
    u = u.reshape(CROPS, CIT, 128, 4, PH * 16).transpose(0, 2, 1, 3, 4)
    u = np.ascontiguousarray(u).reshape(CROPS, 128, CIT * 4 * PH * 16)
    u = u.astype(_BF16)

    in_maps = []
    for h in range(NCORES):
        sl = slice(h * HD, (h + 1) * HD)
        zpad = np.zeros((16,) + wq.shape[1:], wq.dtype)
        w_cat = np.concatenate(
            [wq[sl], wv[sl][:48], wk[sl], wv[sl][48:], zpad], axis=0)  # [256,...]
        # G-transform along kx: laid out [CIT, 128, j, dy, co] = [5, 128, 2304]
        wg = np.einsum('jk,ocdk->cjdo', _G, w_cat.astype(np.float64))
        wg = np.ascontiguousarray(
            wg.reshape(CIT, 128, 4, 3, 256)).reshape(CIT, 128, 4 * 3 * 256)
        b_cat = np.concatenate(
            [bq[sl], bv[sl][:48], bk[sl], bv[sl][48:],
             np.zeros(16, bq.dtype)]).reshape(256, 1)
        wpT = np.ascontiguousarray(w_proj[:, sl].T)  # [80, 640]
        in_maps.append({
            "u": u,
            "wg": wg.astype(_BF16),
            "bqkv": b_cat.astype(np.float32),
            "wproj": wpT.astype(_BF16),
        })
    return in_maps


def _host_reduce(results, b_proj):
    acc = np.zeros((CROPS, C, N), np.float32)
    for r in results:
        acc += r["out"].astype(np.float32) / r["rsum"]
    # un-permute n' = (w-parity, h, w2) back to n = (h, w)
    acc = np.ascontiguousarray(
        acc.reshape(CROPS, C, 2, H, W // 2).transpose(0, 1, 3, 4, 2)
    ).reshape(CROPS, C, N)
    o = acc.reshape(BS, MC, C, N).transpose(0, 3, 1, 2)  # [b, n, m, dout]
    o = o + b_proj[None, None, None, :].astype(np.float32)
    return np.ascontiguousarray(o.reshape(BS, N, MC * C), dtype=np.float32)


_NC_CACHE = {}


def kernel(x, wq, bq, wk, bk, wv, bv, w_proj, b_proj, _run_kwargs=None):
    from concourse.bass_utils import run_bass_kernel_spmd

    if "nc" not in _NC_CACHE:
        _NC_CACHE["nc"] = _build_graph()
    nc = _NC_CACHE["nc"]
    in_maps = _host_inputs(x, wq, bq, wk, bk, wv, bv, w_proj)
    res = run_bass_kernel_spmd(nc, in_maps, core_ids=list(range(NCORES)),
                               **(_run_kwargs or {}))
    out = _host_reduce(res.results, np.asarray(b_proj))
    if _run_kwargs:
        _NC_CACHE["last_result"] = res
    return out
